# revision 1
# baseline (speedup 1.0000x reference)
"""Sparse-weight matmul (BiologicalModule) on 8 Trainium2 NeuronCores.

Computes: out = tanh(x @ scatter_coo(kernel_vector, nonzero_ind) + bias)
  x [32, 30000] f32, 500K COO nonzeros into a [30000, 2048] weight matrix.

Strategy (units-sharded, 256 output columns per core):
  - Never materialize the dense [30000, 2048] weight matrix (245 MB). In CSC
    view, out_T[c, :] = sum_k v[c,k] * x[:, r[c,k]].
  - kernel() packs, per core, a padded-CSC entry payload: for each output
    column its entry values and the x column-vectors those entries touch
    (columns mapped to SBUF partitions; entry slots padded to KP, chunked,
    and stored entry-innermost [col, chunk, batch, k]). This is pure data
    layout / sharding prep - no arithmetic.
  - Each core streams its ~4 MB fp16 payload and does all the math on-chip.
    The entry-innermost layout keeps every tensor_tensor operand 2-byte with
    a step-1 inner dim (the value broadcast is a step-0 *middle* dim), which
    enables the DVE 2x_1P perf mode for the multiply. DVE reduces over the
    entry axis (f32 accumulation); ~1/4 of chunks run multiply + add-tree on
    the otherwise-idle GPSIMD engine (f32 product there); ACT applies fused
    bias + tanh. Chunks overlap DMA / DVE / GPSIMD.
"""

import sys

import numpy as np

_TRN_REPO = "/opt/trn_rl_repo"
if _TRN_REPO not in sys.path:
    sys.path.insert(0, _TRN_REPO)

INPUT_DIM = 30000
UNITS = 2048
BATCH = 32
N_CORES = 8
UNITS_PER_CORE = UNITS // N_CORES  # 256
BLOCKS_PER_CORE = UNITS_PER_CORE // 128  # 2
K_CHUNK = 32  # entry-slots per DMA/compute chunk
# Engine per chunk (cycled): D = DVE mul + add-tree + reduce, A = GPSIMD
# mul + add-tree. 3 of 16 chunks on GPSIMD balances its slower tensor_tensor
# against the 2x-mode DVE path.
ENGINE_PATTERN = "DDDDADDDDADDDDAD"
WORK_BUFS = 8

_PROGRAM_CACHE = {}


def _build_program(kp):
    """Build + compile the SPMD bass program for padded column length kp."""
    from concourse import bacc, tile
    from concourse.bass import AP
    import concourse.mybir as mybir

    assert kp % K_CHUNK == 0
    nch = kp // K_CHUNK
    f32 = mybir.dt.float32
    f16 = mybir.dt.float16

    nc = bacc.Bacc("TRN2", target_bir_lowering=False, debug=False,
                   num_devices=N_CORES)
    g_d = nc.dram_tensor("gvals", [BLOCKS_PER_CORE, 128, nch, BATCH, K_CHUNK],
                         f16, kind="ExternalInput")
    vals_d = nc.dram_tensor("vals", [BLOCKS_PER_CORE, 128, kp], f16,
                            kind="ExternalInput")
    bias_d = nc.dram_tensor("bias2", [128, BLOCKS_PER_CORE], f32,
                            kind="ExternalInput")
    out_d = nc.dram_tensor("out", [BLOCKS_PER_CORE, 128, BATCH], f32,
                           kind="ExternalOutput")

    with tile.TileContext(nc) as tc:
        with (
            tc.tile_pool(name="persist", bufs=1) as persist,
            tc.tile_pool(name="work", bufs=WORK_BUFS) as work,
        ):
            bias_t = persist.tile([128, BLOCKS_PER_CORE], f32, tag="bias")
            nc.sync.dma_start(bias_t[:], bias_d[:])
            gidx = 0
            for blk in range(BLOCKS_PER_CORE):
                val_t = persist.tile([128, kp], f16, tag=f"val{blk}",
                                     name=f"val{blk}")
                nc.sync.dma_start(val_t[:], vals_d[blk])
                pt_t = persist.tile([128, nch, BATCH], f32, tag=f"pt{blk}",
                                    name=f"pt{blk}")
                for ch in range(nch):
                    k0 = ch * K_CHUNK
                    g_t = work.tile([128, BATCH, K_CHUNK], f16, tag="g",
                                    name=f"g{blk}_{ch}")
                    nc.sync.dma_start(g_t[:], g_d[blk, :, ch])
                    use_gp = ENGINE_PATTERN[gidx % len(ENGINE_PATTERN)] == "A"
                    gidx += 1
                    # value operand viewed [p, batch(step 0), k(step 1)]
                    base = val_t[:, k0:k0 + K_CHUNK]
                    v_bk = AP(base.tensor, base.offset,
                              [base.ap[0], [0, BATCH], base.ap[1]])
                    if use_gp:
                        prod = work.tile([128, BATCH, K_CHUNK], f32,
                                         tag="prodA", name=f"prodA{blk}_{ch}")
                        nc.gpsimd.tensor_tensor(prod[:], g_t[:], v_bk,
                                                mybir.AluOpType.mult)
                        w = K_CHUNK
                        while w > 1:
                            h = w // 2
                            nc.gpsimd.tensor_tensor(
                                prod[:, :, :h], prod[:, :, :h],
                                prod[:, :, h:w], mybir.AluOpType.add)
                            w = h
                        nc.gpsimd.tensor_copy(pt_t[:, ch, :], prod[:, :, 0])
                    else:
                        prod = work.tile([128, BATCH, K_CHUNK], f16,
                                         tag="prodD", name=f"prodD{blk}_{ch}")
                        nc.vector.tensor_tensor(prod[:], g_t[:], v_bk,
                                                mybir.AluOpType.mult)
                        # fp16 2x-mode add-tree down to 4 lanes, then a f32
                        # tail reduce for the actual accumulation.
                        with nc.allow_low_precision(
                                "fp16 tree partials; f32 tail reduce"):
                            w = K_CHUNK
                            while w > 4:
                                h = w // 2
                                nc.vector.tensor_tensor(
                                    prod[:, :, :h], prod[:, :, :h],
                                    prod[:, :, h:w], mybir.AluOpType.add)
                                w = h
                        nc.vector.tensor_reduce(
                            pt_t[:, ch, :], prod[:, :, :4],
                            mybir.AxisListType.X, mybir.AluOpType.add)
                red = work.tile([128, BATCH], f32, tag="red", name=f"red{blk}")
                nc.vector.tensor_reduce(
                    red[:],
                    pt_t[:].rearrange("p c b -> p b c"),
                    mybir.AxisListType.X,
                    mybir.AluOpType.add,
                )
                outp = work.tile([128, BATCH], f32, tag="outp",
                                 name=f"outp{blk}")
                nc.scalar.activation(
                    outp[:], red[:], mybir.ActivationFunctionType.Tanh,
                    bias=bias_t[:, blk:blk + 1],
                )
                nc.sync.dma_start(out_d[blk], outp[:])
    nc.compile()
    return nc


def _prepare(x, kernel_vector, bias, nonzero_ind):
    """Host-side shard prep. Returns (kp, per-core input dicts)."""
    x = np.asarray(x, dtype=np.float32)
    v = np.asarray(kernel_vector, dtype=np.float32).ravel()
    bias = np.asarray(bias, dtype=np.float32).ravel()
    ind = np.asarray(nonzero_ind)
    r = ind[:, 0].astype(np.int64)
    c = ind[:, 1].astype(np.int64)

    # COO .set semantics: de-duplicate (row, col), keeping the last occurrence.
    flat = r * UNITS + c
    if len(np.unique(flat)) != len(flat):
        _, last_rev = np.unique(flat[::-1], return_index=True)
        keep = np.sort(len(flat) - 1 - last_rev)
        r, c, v = r[keep], c[keep], v[keep]

    xt16 = np.ascontiguousarray(x.T).astype(np.float16)  # [INPUT_DIM, BATCH]

    # Sort by column, assign each entry its slot k within its column.
    order = np.argsort(c, kind="stable")
    r_s, c_s, v_s = r[order], c[order], v[order]
    counts = np.bincount(c_s, minlength=UNITS)
    kp = max(K_CHUNK, int(-(-counts.max() // K_CHUNK)) * K_CHUNK)
    nch = kp // K_CHUNK
    starts = np.zeros(UNITS + 1, dtype=np.int64)
    np.cumsum(counts, out=starts[1:])
    k_s = np.arange(len(c_s), dtype=np.int64) - starts[c_s]

    # Padded-CSC payload, entry-innermost per chunk: g_all[c, chunk, b, k]
    # holds the x column-vectors the entries touch (fp16); values fp16;
    # products/accumulation are f32 (GPSIMD path) / fp16-product with f32
    # accumulation (DVE path). Padding slots stay 0.
    val_all = np.zeros((UNITS, kp), dtype=np.float16)
    val_all[c_s, k_s] = v_s.astype(np.float16)
    g_all = np.zeros((UNITS, nch, BATCH, K_CHUNK), dtype=np.float16)
    g_all[c_s, k_s // K_CHUNK, :, k_s % K_CHUNK] = xt16[r_s]

    g_all = g_all.reshape(N_CORES, BLOCKS_PER_CORE, 128, nch, BATCH, K_CHUNK)
    val_all = val_all.reshape(N_CORES, BLOCKS_PER_CORE, 128, kp)
    bias2 = np.ascontiguousarray(
        bias.reshape(N_CORES, BLOCKS_PER_CORE, 128).transpose(0, 2, 1))

    in_maps = []
    for d in range(N_CORES):
        in_maps.append({
            "gvals": g_all[d],
            "vals": val_all[d],
            "bias2": bias2[d],
        })
    return kp, in_maps


def _run(inputs, trace=False):
    from concourse.bass_utils import run_bass_kernel_spmd

    kp, in_maps = _prepare(**inputs)
    if kp not in _PROGRAM_CACHE:
        _PROGRAM_CACHE[kp] = _build_program(kp)
    nc = _PROGRAM_CACHE[kp]
    res = None
    for attempt in range(3):
        try:
            res = run_bass_kernel_spmd(
                nc, in_maps, list(range(N_CORES)), trace=trace,
            )
            break
        except Exception:
            # Transient device faults (e.g. NRT_EXEC_UNIT_UNRECOVERABLE)
            # clear on re-execution; re-raise only if persistent.
            if attempt == 2:
                raise
    assert res is not None
    out_t = np.concatenate([res.results[d]["out"].reshape(UNITS_PER_CORE, BATCH)
                            for d in range(N_CORES)], axis=0)  # [2048, 32]
    out = np.ascontiguousarray(out_t.T).astype(np.float32)  # [32, 2048]
    return out, res


def kernel(**inputs):
    out, _ = _run(inputs, trace=False)
    return out



# revision 36
# speedup vs baseline: 1.7685x; 1.7685x over previous
"""Sparse-weight matmul (BiologicalModule) on 8 Trainium2 NeuronCores.

Computes: out = tanh(x @ scatter_coo(kernel_vector, nonzero_ind) + bias)
  x [32, 30000] f32, 500K COO nonzeros into a [30000, 2048] weight matrix.

Strategy (units-sharded, 256 output columns per core, PE-based):
  - Never materialize the dense [30000, 2048] weight matrix. In CSC view,
    out_T[c, :] = sum_k v[c,k] * x[:, r[c,k]].
  - kernel() packs, per core, a slot-major padded-CSC payload: for each
    output column a [K, 32] tile of the x column-vectors its entries touch
    (slot on the partition axis), plus the column's value vector [K, 1].
    Entries are ordered by |v| descending; the first chunk (127 biggest
    entries + a bias slot with G=1) ships in fp16, the low-|v| tail chunk in
    fp8e4 (values prescaled by S=1024 so they stay in e4m3's normal range -
    1/S is folded into the activation scale). This drops the streamed
    payload from 4.2 MB to ~3.1 MB per core while keeping rel err ~1.2e-2.
  - Each column's dot products run on the Tensor engine as an accumulating
    matmul pair: PSUM[c, b] += V_chunk.T @ G_chunk; bias rides in the
    contraction. DVE/GPSIMD (the old bottleneck) do no elementwise work;
    the PE hides entirely under the DMA stream.
  - M=1 matmul outputs can only land on PSUM partitions {0,32,64,96}; ACT
    reads them with a partition-strided AP and applies tanh(x/S) straight
    out of PSUM. Small DMAs ride the GPSIMD SWDGE path so the HWDGE queue
    stays clear for the big fp16 stream.
"""

import sys

import numpy as np

_TRN_REPO = "/opt/trn_rl_repo"
if _TRN_REPO not in sys.path:
    sys.path.insert(0, _TRN_REPO)

import ml_dtypes

from concourse.bass import AP as bass_AP

INPUT_DIM = 30000
UNITS = 2048
BATCH = 32
N_CORES = 8
UNITS_PER_CORE = UNITS // N_CORES  # 256
GROUP_COLS = 64  # columns per PSUM group / G-tile
N_GROUPS = UNITS_PER_CORE // GROUP_COLS  # 8
VSCALE = 1024.0  # value prescale so fp8 values avoid the subnormal range
C0_ENTRIES = 127  # biggest-|v| entries in the fp16 chunk (slot 127 = bias)

_PROGRAM_CACHE = {}


def _chunk_sizes(max_count):
    """Chunk 0: 127 entries + bias (fp16). Rest: <=128-slot fp8 chunks."""
    ks = [128]
    rest = max(0, max_count - C0_ENTRIES)
    while rest > 128:
        ks.append(128)
        rest -= 128
    if rest:
        ks.append(rest)
    return tuple(ks)


# Tapered: small last groups keep the post-stream chain short. Groups are
# the DMA granularity; PSUM/ACT work in 32-column blocks.
COL_GROUPS = [64, 64, 64, 32, 32]
_GSTART = [sum(COL_GROUPS[:i]) for i in range(len(COL_GROUPS))]
BLOCK = 32


def _build_program(ks):
    """Build + compile the SPMD bass program for contraction chunks `ks`."""
    from concourse import bacc, tile
    import concourse.mybir as mybir

    f32 = mybir.dt.float32
    u8 = mybir.dt.uint8
    dts = [mybir.dt.float16] + [mybir.dt.float8e4] * (len(ks) - 1)
    dsz = [2] + [1] * (len(ks) - 1)
    n_chunks = len(ks)

    nc = bacc.Bacc("TRN2", target_bir_lowering=False, debug=False,
                   num_devices=N_CORES)
    g_ds = {}
    for gi, ncols in enumerate(COL_GROUPS):
        for ci, k in enumerate(ks):
            g_ds[gi, ci] = nc.dram_tensor(
                f"g{ci}_{gi}", [k, ncols, BATCH], dts[ci],
                kind="ExternalInput")
    # All value chunks byte-packed into one tensor so a single DMA (the
    # very first bytes on the wire) delivers every stationary operand.
    vbytes = UNITS_PER_CORE * sum(dsz)
    vp_d = nc.dram_tensor("vpack", [128, vbytes], u8, kind="ExternalInput")
    out_d = nc.dram_tensor(
        "out", [128, UNITS_PER_CORE // 128, BATCH], f32,
        kind="ExternalOutput")

    with tile.TileContext(nc) as tc:
        with (
            tc.tile_pool(name="persist", bufs=1) as persist,
            tc.tile_pool(name="work", bufs=1) as work,
            tc.tile_pool(name="fin", bufs=1) as fin,
            tc.psum_pool(name="acc", bufs=1) as acc,
        ):
            # PE p-state warmup source: the cost model reaches the 2.4 GHz
            # p-state only after ~3us of PE activity; dummy matmuls on this
            # zeroed tile keep the PE warm until real work lands.
            warm_t = persist.tile([128, 128], mybir.dt.float16, tag="warm")
            nc.gpsimd.memset(warm_t[:], 0.0)
            wp_t = acc.tile([128, 512], f32, tag="wp")  # full PSUM bank

            def dummy_mms(n):
                for _ in range(n):
                    nc.tensor.matmul(
                        wp_t[0:1, 0:128], warm_t[:, 0:1], warm_t[:],
                        start=True, stop=True, tile_position=(0, 0),
                        skip_group_check=True,
                    )

            # V rides the GPSIMD SWDGE queue: its transfer slots in behind
            # group 0 on the wire but still lands before any matmul needs
            # it, and the big G stream starts one HWDGE latency earlier.
            vp_t = persist.tile([128, vbytes], u8, tag="vp")
            nc.gpsimd.dma_start(vp_t[:], vp_d[:])

            # Zero-spread stationary strips: values land at free offset
            # 32*j (elementwise) with 31 zeros between columns, so column
            # j's stationary is the [K, 32] window at offset 32*j - j%32
            # whose column j%32 is the value vector and the rest are zeros.
            # With M=32 outputs, block b's 32 columns then land on 32
            # *consecutive* PSUM partitions - a fully lane-parallel, legal
            # ACT exit (engines cannot take partition-strided APs).
            vs_ts = []
            for ci, k in enumerate(ks):
                vs_t = persist.tile([k, UNITS_PER_CORE * 32 * dsz[ci]], u8,
                                    tag=f"vs{ci}", name=f"vs{ci}")
                # Zeroing is split across three otherwise-idle engines so
                # it hides under the V/G DMA landing latency (engine memset
                # runs at ~1 elem/cycle regardless of dtype, so use f32
                # views for the fewest elements).
                if ci == 0:
                    half = UNITS_PER_CORE * 16 * dsz[ci]
                    nc.vector.memset(vs_t[:, :half].bitcast(f32), 0)
                    nc.scalar.activation(
                        vs_t[:, half:].bitcast(f32),
                        warm_t[:, 0:1].broadcast_to((k, half // 4)),
                        mybir.ActivationFunctionType.Copy,
                        scale=0.0,
                    )
                else:
                    nc.gpsimd.memset(vs_t[:].bitcast(f32), 0)
                vs_ts.append(vs_t)
            off = 0
            for ci, k in enumerate(ks):
                w = dsz[ci]
                # strided byte-copy: column j's w bytes -> offset 32*w*j
                src = vp_t[0:k, off:off + UNITS_PER_CORE * w]
                dst = bass_AP(vs_ts[ci].tensor, 0,
                              [vs_ts[ci][:].ap[0],
                               [32 * w, UNITS_PER_CORE], [1, w]])
                nc.vector.tensor_copy(dst, src)
                off += UNITS_PER_CORE * w

            g_tiles = []
            for gi, ncols in enumerate(COL_GROUPS):
                g_ts = []
                for ci, k in enumerate(ks):
                    g_t = work.tile([k, ncols, BATCH], dts[ci],
                                    tag=f"g{ci}w{ncols}", name=f"g{ci}_{gi}",
                                    bufs=3 if ncols == 64 else 2)
                    nc.sync.dma_start(g_t[:], g_ds[gi, ci][:])
                    g_ts.append(g_t)
                g_tiles.append(g_ts)

            # fin: block b -> partitions (b%4)*32..+31, sliver b//4.
            fin_t = fin.tile([128, UNITS_PER_CORE // 128, BATCH], f32,
                             tag="fin")
            dummy_mms(40)
            for gi, ncols in enumerate(COL_GROUPS):
                g_ts = g_tiles[gi]
                # Phase-ordered per block: all chunk-0 matmuls, then all
                # chunk-1 matmuls. Only the chunk that lands last gates its
                # own phase (not the whole block), and only the block's
                # very first matmul carries start=True (one has_written
                # clear per PSUM bank; later rows overwrite-as-virgin).
                for b0 in range(_GSTART[gi] // BLOCK,
                               (_GSTART[gi] + ncols) // BLOCK):
                    pb, sl = (b0 % 4) * 32, b0 // 4
                    p_t = acc.tile([128, 512], f32, tag="pb",
                                   name=f"p{b0}", bufs=4)
                    for ci in range(n_chunks):
                        w = dsz[ci]
                        for m in range(BLOCK):
                            j = b0 * BLOCK + m
                            vs = vs_ts[ci][:, (BLOCK * j - m) * w:
                                           (BLOCK * (j + 1) - m) * w]
                            nc.tensor.matmul(
                                p_t[pb:pb + 32, 0:BATCH],
                                vs.bitcast(dts[ci]),
                                g_ts[ci][:, j - _GSTART[gi], :],
                                start=(m == 0 and ci == 0),
                                stop=(m == BLOCK - 1 and ci == n_chunks - 1),
                                tile_position=(0, pb),
                            )
                    # Lane-parallel fused tanh(psum / VSCALE) from PSUM.
                    nc.scalar.activation(
                        fin_t[pb:pb + 32, sl], p_t[pb:pb + 32, 0:BATCH],
                        mybir.ActivationFunctionType.Tanh,
                        scale=1.0 / VSCALE,
                    )
            # One output DMA: it waits only on the final block's ACT.
            nc.scalar.dma_start(out_d[:], fin_t[:])
    nc.compile()
    return nc


def _prepare(x, kernel_vector, bias, nonzero_ind):
    """Host-side shard prep. Returns (ks, per-core input dicts)."""
    x = np.asarray(x, dtype=np.float32)
    v = np.asarray(kernel_vector, dtype=np.float32).ravel()
    bias = np.asarray(bias, dtype=np.float32).ravel()
    ind = np.asarray(nonzero_ind)
    r = ind[:, 0].astype(np.int64)
    c = ind[:, 1].astype(np.int64)

    # COO .set semantics: de-duplicate (row, col), keeping the last occurrence.
    flat = r * UNITS + c
    if len(np.unique(flat)) != len(flat):
        _, last_rev = np.unique(flat[::-1], return_index=True)
        keep = np.sort(len(flat) - 1 - last_rev)
        r, c, v = r[keep], c[keep], v[keep]

    xt16 = np.ascontiguousarray(x.T).astype(np.float16)  # [INPUT_DIM, BATCH]

    # Sort by (column, |v| desc); slot k within column = |v| rank.
    order = np.lexsort((-np.abs(v), c))
    r_s, c_s, v_s = r[order], c[order], v[order]
    counts = np.bincount(c_s, minlength=UNITS)
    ks = _chunk_sizes(int(counts.max()))
    kp = 1 + C0_ENTRIES + sum(ks[1:])  # dense slot space incl bias at 127
    starts = np.zeros(UNITS + 1, dtype=np.int64)
    np.cumsum(counts, out=starts[1:])
    k_s = np.arange(len(c_s), dtype=np.int64) - starts[c_s]
    # entry slot: rank<127 -> slot=rank (chunk 0); else slot=rank+1
    slot = np.where(k_s < C0_ENTRIES, k_s, k_s + 1)

    vs_scaled = (v_s * VSCALE).astype(np.float32)
    val_all = np.zeros((UNITS, kp), dtype=np.float32)
    val_all[c_s, slot] = vs_scaled
    val_all[:, C0_ENTRIES] = bias * VSCALE
    g_all = np.zeros((UNITS, kp, BATCH), dtype=np.float16)
    g_all[c_s, slot] = xt16[r_s]
    g_all[:, C0_ENTRIES] = 1.0

    g_all = g_all.reshape(N_CORES, UNITS_PER_CORE, kp, BATCH)
    val_all = val_all.reshape(N_CORES, UNITS_PER_CORE, kp)

    f8 = ml_dtypes.float8_e4m3
    np_dts = [np.float16] + [f8] * (len(ks) - 1)
    dsz = [2] + [1] * (len(ks) - 1)
    vbytes = UNITS_PER_CORE * sum(dsz)
    in_maps = []
    for d in range(N_CORES):
        m = {}
        vpack = np.zeros((128, vbytes), dtype=np.uint8)
        off = 0
        boff = 0
        for ci, k in enumerate(ks):
            gc = g_all[d, :, off:off + k].astype(np_dts[ci])  # [col, k, b]
            for gi, ncols in enumerate(COL_GROUPS):
                cs = slice(_GSTART[gi], _GSTART[gi] + ncols)
                m[f"g{ci}_{gi}"] = np.ascontiguousarray(
                    gc[cs].transpose(1, 0, 2))
            vchunk = np.ascontiguousarray(
                val_all[d, :, off:off + k].T).astype(np_dts[ci])
            w = UNITS_PER_CORE * dsz[ci]
            vpack[:k, boff:boff + w] = vchunk.view(np.uint8)
            off += k
            boff += w
        m["vpack"] = vpack
        in_maps.append(m)
    return ks, in_maps


def _unscramble(res):
    """[core][part, sliver, b] -> [32, 2048] f32.

    Column j sits at partition ((j//32)%4)*32 + j%32, sliver j//128.
    """
    n_sl = UNITS_PER_CORE // 128
    parts = np.arange(128)
    slivers = np.arange(n_sl)
    # j[p, sl] = 32*(4*sl + p//32) + p%32
    jmap = (32 * (4 * slivers[None, :] + parts[:, None] // 32)
            + (parts[:, None] % 32))
    out = np.empty((UNITS, BATCH), dtype=np.float32)
    for d in range(N_CORES):
        o = res.results[d]["out"].reshape(128, n_sl, BATCH)
        cols = d * UNITS_PER_CORE
        out[cols + jmap.ravel()] = o.reshape(128 * n_sl, BATCH)
    return np.ascontiguousarray(out.T)


def _run(inputs, trace=False):
    from concourse.bass_utils import run_bass_kernel_spmd

    ks, in_maps = _prepare(**inputs)
    if ks not in _PROGRAM_CACHE:
        _PROGRAM_CACHE[ks] = _build_program(ks)
    nc = _PROGRAM_CACHE[ks]
    res = None
    for attempt in range(3):
        try:
            res = run_bass_kernel_spmd(
                nc, in_maps, list(range(N_CORES)), trace=trace,
            )
            break
        except Exception:
            # Transient device faults (e.g. NRT_EXEC_UNIT_UNRECOVERABLE)
            # clear on re-execution; re-raise only if persistent.
            if attempt == 2:
                raise
    assert res is not None
    return _unscramble(res), res


def kernel(**inputs):
    out, _ = _run(inputs, trace=False)
    return out


# revision 41
# speedup vs baseline: 1.8278x; 1.0335x over previous
"""Sparse-weight matmul (BiologicalModule) on 8 Trainium2 NeuronCores.

Computes: out = tanh(x @ scatter_coo(kernel_vector, nonzero_ind) + bias)
  x [32, 30000] f32, 500K COO nonzeros into a [30000, 2048] weight matrix.

Strategy (units-sharded, 256 output columns per core, PE-based):
  - Never materialize the dense [30000, 2048] weight matrix. In CSC view,
    out_T[c, :] = sum_k v[c,k] * x[:, r[c,k]].
  - kernel() packs, per core, a slot-major padded-CSC payload: for each
    output column a [K, 32] tile of the x column-vectors its entries touch
    (slot on the partition axis), plus the column's value vector [K, 1].
    Entries are ordered by |v| descending; the first chunk (127 biggest
    entries + a bias slot with G=1) ships in fp16, the low-|v| tail chunk in
    fp8e4 (values prescaled by S=1024 so they stay in e4m3's normal range -
    1/S is folded into the activation scale). This drops the streamed
    payload from 4.2 MB to ~3.1 MB per core while keeping rel err ~1.2e-2.
  - Each column's dot products run on the Tensor engine as an accumulating
    matmul pair: PSUM[c, b] += V_chunk.T @ G_chunk; bias rides in the
    contraction. DVE/GPSIMD (the old bottleneck) do no elementwise work;
    the PE hides entirely under the DMA stream.
  - M=1 matmul outputs can only land on PSUM partitions {0,32,64,96}; ACT
    reads them with a partition-strided AP and applies tanh(x/S) straight
    out of PSUM. Small DMAs ride the GPSIMD SWDGE path so the HWDGE queue
    stays clear for the big fp16 stream.
"""

import sys

import numpy as np

_TRN_REPO = "/opt/trn_rl_repo"
if _TRN_REPO not in sys.path:
    sys.path.insert(0, _TRN_REPO)

import ml_dtypes

from concourse.bass import AP as bass_AP

INPUT_DIM = 30000
UNITS = 2048
BATCH = 32
N_CORES = 8
UNITS_PER_CORE = UNITS // N_CORES  # 256
GROUP_COLS = 64  # columns per PSUM group / G-tile
N_GROUPS = UNITS_PER_CORE // GROUP_COLS  # 8
VSCALE = 1024.0  # value prescale so fp8 values avoid the subnormal range
C0_ENTRIES = 127  # biggest-|v| entries in the fp16 chunk (slot 127 = bias)

_PROGRAM_CACHE = {}


def _chunk_sizes(max_count):
    """Chunk 0: 127 entries + bias (fp16). Rest: <=128-slot fp8 chunks."""
    ks = [128]
    rest = max(0, max_count - C0_ENTRIES)
    while rest > 128:
        ks.append(128)
        rest -= 128
    if rest:
        ks.append(rest)
    return tuple(ks)


# Tapered: small last groups keep the post-stream chain short. Groups are
# the DMA granularity; PSUM/ACT work in 32-column blocks.
COL_GROUPS = [64, 64, 64, 32, 32]
_GSTART = [sum(COL_GROUPS[:i]) for i in range(len(COL_GROUPS))]
BLOCK = 32


def _build_program(ks):
    """Build + compile the SPMD bass program for contraction chunks `ks`."""
    from concourse import bacc, tile
    import concourse.mybir as mybir

    f32 = mybir.dt.float32
    u8 = mybir.dt.uint8
    f16 = mybir.dt.float16
    f8 = mybir.dt.float8e4
    assert len(ks) == 2
    KD = (ks[1] + 1) // 2  # fp8 tail as two k-tiles, contracted by one
    # DoubleRow matmul at 0.5 cycles/row

    nc = bacc.Bacc("TRN2", target_bir_lowering=False, debug=False,
                   num_devices=N_CORES)
    g_ds = {}
    for gi, ncols in enumerate(COL_GROUPS):
        g_ds[gi, 0] = nc.dram_tensor(
            f"g0_{gi}", [ks[0], ncols, BATCH], f16, kind="ExternalInput")
        g_ds[gi, 1] = nc.dram_tensor(
            f"g1_{gi}", [KD, 2, ncols, BATCH], f8, kind="ExternalInput")
    # All value chunks byte-packed into one tensor so a single DMA (the
    # very first bytes on the wire) delivers every stationary operand.
    vbytes = UNITS_PER_CORE * 4  # 2B fp16 + 2 ktiles x 1B fp8
    vp_d = nc.dram_tensor("vpack", [128, vbytes], u8, kind="ExternalInput")
    out_d = nc.dram_tensor(
        "out", [32, UNITS_PER_CORE // 32, BATCH], f32,
        kind="ExternalOutput")

    with tile.TileContext(nc) as tc:
        with (
            tc.tile_pool(name="persist", bufs=1) as persist,
            tc.tile_pool(name="work", bufs=1) as work,
            tc.tile_pool(name="fin", bufs=1) as fin,
            tc.psum_pool(name="acc", bufs=1) as acc,
        ):
            # PE p-state warmup source: the cost model reaches the 2.4 GHz
            # p-state only after ~3us of PE activity; dummy matmuls on this
            # zeroed tile keep the PE warm until real work lands.
            warm_t = persist.tile([128, 128], mybir.dt.float16, tag="warm")
            nc.gpsimd.memset(warm_t[:], 0.0)
            wp_t = acc.tile([128, 512], f32, tag="wp")  # full PSUM bank

            def dummy_mms(n):
                for _ in range(n):
                    nc.tensor.matmul(
                        wp_t[0:1, 0:128], warm_t[:, 0:1], warm_t[:],
                        start=True, stop=True, tile_position=(0, 0),
                        skip_group_check=True,
                    )

            # V rides the GPSIMD SWDGE queue: its transfer slots in behind
            # group 0 on the wire but still lands before any matmul needs
            # it, and the big G stream starts one HWDGE latency earlier.
            vp_t = persist.tile([128, vbytes], u8, tag="vp")
            nc.gpsimd.dma_start(vp_t[:], vp_d[:])

            # Zero-spread stationary strips: values land at free offset
            # 32*j (elementwise) with 31 zeros between columns, so column
            # j's stationary is the [K, 32] window at offset 32*j - j%32
            # whose column j%32 is the value vector and the rest are zeros.
            # With M=32 outputs, block b's 32 columns then land on 32
            # *consecutive* PSUM partitions - a fully lane-parallel, legal
            # ACT exit (engines cannot take partition-strided APs).
            vs0_t = persist.tile([ks[0], UNITS_PER_CORE * 32 * 2], u8,
                                 tag="vs0")
            vs1_t = persist.tile([KD, 2, UNITS_PER_CORE * 32], u8,
                                 tag="vs1")
            # Zeroing is split across three otherwise-idle engines so it
            # hides under the V/G DMA landing latency (engine memsets run
            # at ~1 elem/cycle regardless of dtype, so use f32 views for
            # the fewest elements). ACT's shares sit behind the ~1.3us
            # tanh-table load, so they are the smaller cuts.
            v0f = vs0_t[:].bitcast(f32)   # [128, 4096]
            v1f = vs1_t[:].bitcast(f32)   # [KD, 2, 2048]
            nc.vector.memset(v0f[:, :2868], 0)
            nc.scalar.activation(
                v0f[:, 2868:],
                warm_t[:, 0:1].broadcast_to((ks[0], 4096 - 2868)),
                mybir.ActivationFunctionType.Copy, scale=0.0)
            nc.gpsimd.memset(v1f[:, :, :1434], 0)
            nc.scalar.activation(
                v1f[:, :, 1434:],
                warm_t[0:KD, 0:1].broadcast_to((KD, 2, 2048 - 1434)),
                mybir.ActivationFunctionType.Copy, scale=0.0)
            # strided byte-copies: column j's bytes -> elem offset 32*j
            sp0_dst = bass_AP(vs0_t.tensor, 0,
                              [vs0_t[:].ap[0], [64, UNITS_PER_CORE], [1, 2]])
            nc.vector.tensor_copy(sp0_dst, vp_t[0:ks[0], 0:2 * UNITS_PER_CORE])
            sp1_src = vp_t[0:KD, 2 * UNITS_PER_CORE:].bitcast(u8)
            nc.vector.tensor_copy(
                vs1_t[:, :, 0:UNITS_PER_CORE * 32:32],
                bass_AP(sp1_src.tensor, sp1_src.offset,
                        [sp1_src.ap[0], [UNITS_PER_CORE, 2],
                         [1, UNITS_PER_CORE]]))

            g_tiles = []
            for gi, ncols in enumerate(COL_GROUPS):
                g_ts = []
                shapes = [([ks[0], ncols, BATCH], f16),
                          ([KD, 2, ncols, BATCH], f8)]
                for ci, (shp, dt_) in enumerate(shapes):
                    g_t = work.tile(shp, dt_,
                                    tag=f"g{ci}w{ncols}", name=f"g{ci}_{gi}",
                                    bufs=3 if ncols == 64 else 2)
                    cdim = len(shp) - 2
                    if gi == len(COL_GROUPS) - 1:
                        # Halved loads: the final DMA (and its +900ns sem
                        # propagation) gates only 16 columns' matmuls.
                        h = ncols // 2
                        if ci == 0:
                            nc.sync.dma_start(g_t[:, :h, :],
                                              g_ds[gi, ci][:, :h, :])
                            nc.sync.dma_start(g_t[:, h:, :],
                                              g_ds[gi, ci][:, h:, :])
                        else:
                            nc.sync.dma_start(g_t[:, :, :h, :],
                                              g_ds[gi, ci][:, :, :h, :])
                            nc.sync.dma_start(g_t[:, :, h:, :],
                                              g_ds[gi, ci][:, :, h:, :])
                    else:
                        nc.sync.dma_start(g_t[:], g_ds[gi, ci][:])
                    g_ts.append(g_t)
                g_tiles.append(g_ts)

            # fin: block b -> partitions 0..31 (DoubleRow matmuls may only
            # target PSUM partition base 0), sliver b.
            fin_t = fin.tile([32, UNITS_PER_CORE // 32, BATCH], f32,
                             tag="fin")
            dummy_mms(40)
            for gi, ncols in enumerate(COL_GROUPS):
                g_ts = g_tiles[gi]
                # Phase-ordered per block: all chunk-0 matmuls, then all
                # chunk-1 matmuls. Only the chunk that lands last gates its
                # own phase (not the whole block), and only the block's
                # very first matmul carries start=True (one has_written
                # clear per PSUM bank; later rows overwrite-as-virgin).
                for b0 in range(_GSTART[gi] // BLOCK,
                               (_GSTART[gi] + ncols) // BLOCK):
                    pb = 0
                    p_t = acc.tile([128, 512], f32, tag="pb",
                                   name=f"p{b0}", bufs=4)
                    for m in range(BLOCK):
                        j = b0 * BLOCK + m
                        vs = vs0_t[:, (BLOCK * j - m) * 2:
                                   (BLOCK * (j + 1) - m) * 2]
                        nc.tensor.matmul(
                            p_t[pb:pb + 32, 0:BATCH],
                            vs.bitcast(f16),
                            g_ts[0][:, j - _GSTART[gi], :],
                            start=(m == 0),
                            stop=False,
                            tile_position=(0, pb),
                        )
                    for m in range(BLOCK):
                        j = b0 * BLOCK + m
                        vs = vs1_t[:, :, BLOCK * j - m:BLOCK * (j + 1) - m]
                        nc.tensor.matmul(
                            p_t[pb:pb + 32, 0:BATCH],
                            vs.bitcast(f8),
                            g_ts[1][:, :, j - _GSTART[gi], :],
                            start=False,
                            stop=(m == BLOCK - 1),
                            perf_mode=mybir.MatmulPerfMode.DoubleRow,
                            tile_position=(0, pb),
                        )
                    # Lane-parallel fused tanh(psum / VSCALE) from PSUM.
                    nc.scalar.activation(
                        fin_t[:, b0], p_t[0:32, 0:BATCH],
                        mybir.ActivationFunctionType.Tanh,
                        scale=1.0 / VSCALE,
                    )
            # One output DMA: it waits only on the final block's ACT.
            nc.scalar.dma_start(out_d[:], fin_t[:])
    nc.compile()
    return nc


def _prepare(x, kernel_vector, bias, nonzero_ind):
    """Host-side shard prep. Returns (ks, per-core input dicts)."""
    x = np.asarray(x, dtype=np.float32)
    v = np.asarray(kernel_vector, dtype=np.float32).ravel()
    bias = np.asarray(bias, dtype=np.float32).ravel()
    ind = np.asarray(nonzero_ind)
    r = ind[:, 0].astype(np.int64)
    c = ind[:, 1].astype(np.int64)

    # COO .set semantics: de-duplicate (row, col), keeping the last occurrence.
    flat = r * UNITS + c
    if len(np.unique(flat)) != len(flat):
        _, last_rev = np.unique(flat[::-1], return_index=True)
        keep = np.sort(len(flat) - 1 - last_rev)
        r, c, v = r[keep], c[keep], v[keep]

    xt16 = np.ascontiguousarray(x.T).astype(np.float16)  # [INPUT_DIM, BATCH]

    # Sort by (column, |v| desc); slot k within column = |v| rank.
    order = np.lexsort((-np.abs(v), c))
    r_s, c_s, v_s = r[order], c[order], v[order]
    counts = np.bincount(c_s, minlength=UNITS)
    ks = _chunk_sizes(int(counts.max()))
    kp = 1 + C0_ENTRIES + sum(ks[1:])  # dense slot space incl bias at 127
    starts = np.zeros(UNITS + 1, dtype=np.int64)
    np.cumsum(counts, out=starts[1:])
    k_s = np.arange(len(c_s), dtype=np.int64) - starts[c_s]
    # entry slot: rank<127 -> slot=rank (chunk 0); else slot=rank+1
    slot = np.where(k_s < C0_ENTRIES, k_s, k_s + 1)

    vs_scaled = (v_s * VSCALE).astype(np.float32)
    val_all = np.zeros((UNITS, kp), dtype=np.float32)
    val_all[c_s, slot] = vs_scaled
    val_all[:, C0_ENTRIES] = bias * VSCALE
    g_all = np.zeros((UNITS, kp, BATCH), dtype=np.float16)
    g_all[c_s, slot] = xt16[r_s]
    g_all[:, C0_ENTRIES] = 1.0

    g_all = g_all.reshape(N_CORES, UNITS_PER_CORE, kp, BATCH)
    val_all = val_all.reshape(N_CORES, UNITS_PER_CORE, kp)

    f8 = ml_dtypes.float8_e4m3
    assert len(ks) == 2
    KD = (ks[1] + 1) // 2
    vbytes = UNITS_PER_CORE * 4
    in_maps = []
    for d in range(N_CORES):
        m = {}
        vpack = np.zeros((128, vbytes), dtype=np.uint8)
        # fp16 chunk
        gc0 = g_all[d, :, :ks[0]].astype(np.float16)  # [col, k, b]
        # fp8 tail, zero-padded to 2*KD slots, as [col, ktile, KD, b]
        gc1 = np.zeros((UNITS_PER_CORE, 2 * KD, BATCH), np.float32)
        gc1[:, :ks[1]] = g_all[d, :, ks[0]:].astype(np.float32)
        gc1 = gc1.reshape(UNITS_PER_CORE, 2, KD, BATCH).astype(f8)
        for gi, ncols in enumerate(COL_GROUPS):
            cs = slice(_GSTART[gi], _GSTART[gi] + ncols)
            m[f"g0_{gi}"] = np.ascontiguousarray(gc0[cs].transpose(1, 0, 2))
            m[f"g1_{gi}"] = np.ascontiguousarray(
                gc1[cs].transpose(2, 1, 0, 3))
        v0 = np.ascontiguousarray(
            val_all[d, :, :ks[0]].T).astype(np.float16)
        vpack[:ks[0], :2 * UNITS_PER_CORE] = v0.view(np.uint8)
        v1 = np.zeros((UNITS_PER_CORE, 2 * KD), np.float32)
        v1[:, :ks[1]] = val_all[d, :, ks[0]:]
        v1 = v1.reshape(UNITS_PER_CORE, 2, KD).astype(f8)
        # [col, kt, pos] -> [pos, kt, col]
        vpack[:KD, 2 * UNITS_PER_CORE:] = np.ascontiguousarray(
            v1.transpose(2, 1, 0)).reshape(KD, 2 * UNITS_PER_CORE).view(
                np.uint8)
        m["vpack"] = vpack
        in_maps.append(m)
    return ks, in_maps


def _unscramble(res):
    """[core][part, block, b] -> [32, 2048] f32. Column j at [j%32, j//32]."""
    nblk = UNITS_PER_CORE // 32
    out = np.empty((UNITS, BATCH), dtype=np.float32)
    jmap = (np.arange(32)[:, None] + 32 * np.arange(nblk)[None, :])
    for d in range(N_CORES):
        o = res.results[d]["out"].reshape(32, nblk, BATCH)
        out[d * UNITS_PER_CORE + jmap.ravel()] = o.reshape(-1, BATCH)
    return np.ascontiguousarray(out.T)


def _run(inputs, trace=False):
    from concourse.bass_utils import run_bass_kernel_spmd

    ks, in_maps = _prepare(**inputs)
    if ks not in _PROGRAM_CACHE:
        _PROGRAM_CACHE[ks] = _build_program(ks)
    nc = _PROGRAM_CACHE[ks]
    res = None
    for attempt in range(3):
        try:
            res = run_bass_kernel_spmd(
                nc, in_maps, list(range(N_CORES)), trace=trace,
            )
            break
        except Exception:
            # Transient device faults (e.g. NRT_EXEC_UNIT_UNRECOVERABLE)
            # clear on re-execution; re-raise only if persistent.
            if attempt == 2:
                raise
    assert res is not None
    return _unscramble(res), res


def kernel(**inputs):
    out, _ = _run(inputs, trace=False)
    return out


# revision 44
# speedup vs baseline: 1.8657x; 1.0207x over previous
"""Sparse-weight matmul (BiologicalModule) on 8 Trainium2 NeuronCores.

Computes: out = tanh(x @ scatter_coo(kernel_vector, nonzero_ind) + bias)
  x [32, 30000] f32, 500K COO nonzeros into a [30000, 2048] weight matrix.

Strategy (units-sharded, 256 output columns per core, PE-based):
  - Never materialize the dense [30000, 2048] weight matrix. In CSC view,
    out_T[c, :] = sum_k v[c,k] * x[:, r[c,k]].
  - kernel() packs, per core, a slot-major padded-CSC payload: for each
    output column a [K, 32] tile of the x column-vectors its entries touch
    (slot on the partition axis), plus the column's value vector [K, 1].
    Entries are ordered by |v| descending; the first chunk (127 biggest
    entries + a bias slot with G=1) ships in fp16, the low-|v| tail chunk in
    fp8e4 (values prescaled by S=1024 so they stay in e4m3's normal range -
    1/S is folded into the activation scale). This drops the streamed
    payload from 4.2 MB to ~3.1 MB per core while keeping rel err ~1.2e-2.
  - Each column's dot products run on the Tensor engine as an accumulating
    matmul pair: PSUM[c, b] += V_chunk.T @ G_chunk; bias rides in the
    contraction. DVE/GPSIMD (the old bottleneck) do no elementwise work;
    the PE hides entirely under the DMA stream.
  - M=1 matmul outputs can only land on PSUM partitions {0,32,64,96}; ACT
    reads them with a partition-strided AP and applies tanh(x/S) straight
    out of PSUM. Small DMAs ride the GPSIMD SWDGE path so the HWDGE queue
    stays clear for the big fp16 stream.
"""

import sys

import numpy as np

_TRN_REPO = "/opt/trn_rl_repo"
if _TRN_REPO not in sys.path:
    sys.path.insert(0, _TRN_REPO)

import ml_dtypes

from concourse.bass import AP as bass_AP

INPUT_DIM = 30000
UNITS = 2048
BATCH = 32
N_CORES = 8
UNITS_PER_CORE = UNITS // N_CORES  # 256
GROUP_COLS = 64  # columns per PSUM group / G-tile
N_GROUPS = UNITS_PER_CORE // GROUP_COLS  # 8
VSCALE = 1024.0  # value prescale so fp8 values avoid the subnormal range
C0_ENTRIES = 112  # biggest-|v| entries in the fp16 chunk (last slot = bias)

_PROGRAM_CACHE = {}


def _chunk_sizes(max_count):
    """Chunk 0: C0_ENTRIES entries + bias (fp16); chunk 1: the fp8 tail."""
    return (C0_ENTRIES + 1, max(1, max_count - C0_ENTRIES))


# Tapered: small last groups keep the post-stream chain short. Groups are
# the DMA granularity; PSUM/ACT work in 32-column blocks.
COL_GROUPS = [64, 64, 64, 32, 32]
_GSTART = [sum(COL_GROUPS[:i]) for i in range(len(COL_GROUPS))]
BLOCK = 32


def _build_program(ks):
    """Build + compile the SPMD bass program for contraction chunks `ks`."""
    from concourse import bacc, tile
    import concourse.mybir as mybir

    f32 = mybir.dt.float32
    u8 = mybir.dt.uint8
    f16 = mybir.dt.float16
    f8 = mybir.dt.float8e4
    assert len(ks) == 2
    KD = (ks[1] + 1) // 2  # fp8 tail as two k-tiles, contracted by one
    # DoubleRow matmul at 0.5 cycles/row

    nc = bacc.Bacc("TRN2", target_bir_lowering=False, debug=False,
                   num_devices=N_CORES)
    g_ds = {}
    for gi, ncols in enumerate(COL_GROUPS):
        g_ds[gi, 0] = nc.dram_tensor(
            f"g0_{gi}", [ks[0], ncols, BATCH], f16, kind="ExternalInput")
        g_ds[gi, 1] = nc.dram_tensor(
            f"g1_{gi}", [KD, 2, ncols, BATCH], f8, kind="ExternalInput")
    # All value chunks byte-packed into one tensor so a single DMA (the
    # very first bytes on the wire) delivers every stationary operand.
    vbytes = UNITS_PER_CORE * 4  # 2B fp16 + 2 ktiles x 1B fp8
    vp_d = nc.dram_tensor("vpack", [128, vbytes], u8, kind="ExternalInput")
    out_d = nc.dram_tensor(
        "out", [32, UNITS_PER_CORE // 32, BATCH], f32,
        kind="ExternalOutput")

    with tile.TileContext(nc) as tc:
        with (
            tc.tile_pool(name="persist", bufs=1) as persist,
            tc.tile_pool(name="work", bufs=1) as work,
            tc.tile_pool(name="fin", bufs=1) as fin,
            tc.psum_pool(name="acc", bufs=1) as acc,
        ):
            # PE p-state warmup source: the cost model reaches the 2.4 GHz
            # p-state only after ~3us of PE activity; dummy matmuls on this
            # zeroed tile keep the PE warm until real work lands.
            warm_t = persist.tile([128, 128], mybir.dt.float16, tag="warm")
            nc.gpsimd.memset(warm_t[:], 0.0)
            wp_t = acc.tile([128, 512], f32, tag="wp")  # full PSUM bank

            def dummy_mms(n):
                for _ in range(n):
                    nc.tensor.matmul(
                        wp_t[0:1, 0:128], warm_t[:, 0:1], warm_t[:],
                        start=True, stop=True, tile_position=(0, 0),
                        skip_group_check=True,
                    )

            # V rides the GPSIMD SWDGE queue: its transfer slots in behind
            # group 0 on the wire but still lands before any matmul needs
            # it, and the big G stream starts one HWDGE latency earlier.
            vp_t = persist.tile([128, vbytes], u8, tag="vp")
            nc.gpsimd.dma_start(vp_t[:], vp_d[:])

            # Zero-spread stationary strips: values land at free offset
            # 32*j (elementwise) with 31 zeros between columns, so column
            # j's stationary is the [K, 32] window at offset 32*j - j%32
            # whose column j%32 is the value vector and the rest are zeros.
            # With M=32 outputs, block b's 32 columns then land on 32
            # *consecutive* PSUM partitions - a fully lane-parallel, legal
            # ACT exit (engines cannot take partition-strided APs).
            vs0_t = persist.tile([ks[0], UNITS_PER_CORE * 32 * 2], u8,
                                 tag="vs0")
            vs1_t = persist.tile([KD, 2, UNITS_PER_CORE * 32], u8,
                                 tag="vs1")
            # Zeroing is split across three otherwise-idle engines so it
            # hides under the V/G DMA landing latency (engine memsets run
            # at ~1 elem/cycle regardless of dtype, so use f32 views for
            # the fewest elements). ACT's shares sit behind the ~1.3us
            # tanh-table load, so they are the smaller cuts.
            v0f = vs0_t[:].bitcast(f32)   # [128, 4096]
            v1f = vs1_t[:].bitcast(f32)   # [KD, 2, 2048]
            nc.vector.memset(v0f[:, :2868], 0)
            nc.scalar.activation(
                v0f[:, 2868:],
                warm_t[0:ks[0], 0:1].broadcast_to((ks[0], 4096 - 2868)),
                mybir.ActivationFunctionType.Copy, scale=0.0)
            nc.gpsimd.memset(v1f[:, :, :1434], 0)
            nc.scalar.activation(
                v1f[:, :, 1434:],
                warm_t[0:KD, 0:1].broadcast_to((KD, 2, 2048 - 1434)),
                mybir.ActivationFunctionType.Copy, scale=0.0)
            # strided byte-copies: column j's bytes -> elem offset 32*j
            sp0_dst = bass_AP(vs0_t.tensor, 0,
                              [vs0_t[:].ap[0], [64, UNITS_PER_CORE], [1, 2]])
            nc.vector.tensor_copy(sp0_dst, vp_t[0:ks[0], 0:2 * UNITS_PER_CORE])
            sp1_src = vp_t[0:KD, 2 * UNITS_PER_CORE:].bitcast(u8)
            nc.vector.tensor_copy(
                vs1_t[:, :, 0:UNITS_PER_CORE * 32:32],
                bass_AP(sp1_src.tensor, sp1_src.offset,
                        [sp1_src.ap[0], [UNITS_PER_CORE, 2],
                         [1, UNITS_PER_CORE]]))

            g_tiles = []
            for gi, ncols in enumerate(COL_GROUPS):
                g_ts = []
                shapes = [([ks[0], ncols, BATCH], f16),
                          ([KD, 2, ncols, BATCH], f8)]
                for ci, (shp, dt_) in enumerate(shapes):
                    g_t = work.tile(shp, dt_,
                                    tag=f"g{ci}w{ncols}", name=f"g{ci}_{gi}",
                                    bufs=3 if ncols == 64 else 2)
                    if gi == len(COL_GROUPS) - 1:
                        # Halved loads: the final DMA (and its +900ns sem
                        # propagation) gates only 16 columns' matmuls.
                        h = ncols // 2
                        if ci == 0:
                            nc.sync.dma_start(g_t[:, :h, :],
                                              g_ds[gi, ci][:, :h, :])
                            nc.sync.dma_start(g_t[:, h:, :],
                                              g_ds[gi, ci][:, h:, :])
                        else:
                            nc.sync.dma_start(g_t[:, :, :h, :],
                                              g_ds[gi, ci][:, :, :h, :])
                            nc.sync.dma_start(g_t[:, :, h:, :],
                                              g_ds[gi, ci][:, :, h:, :])
                    else:
                        nc.sync.dma_start(g_t[:], g_ds[gi, ci][:])
                    g_ts.append(g_t)
                g_tiles.append(g_ts)

            # fin: block b -> partitions 0..31 (DoubleRow matmuls may only
            # target PSUM partition base 0), sliver b.
            fin_t = fin.tile([32, UNITS_PER_CORE // 32, BATCH], f32,
                             tag="fin")
            dummy_mms(40)
            for gi, ncols in enumerate(COL_GROUPS):
                g_ts = g_tiles[gi]
                # Phase-ordered per block: all chunk-0 matmuls, then all
                # chunk-1 matmuls. Only the chunk that lands last gates its
                # own phase (not the whole block), and only the block's
                # very first matmul carries start=True (one has_written
                # clear per PSUM bank; later rows overwrite-as-virgin).
                for b0 in range(_GSTART[gi] // BLOCK,
                               (_GSTART[gi] + ncols) // BLOCK):
                    pb = 0
                    p_t = acc.tile([128, 512], f32, tag="pb",
                                   name=f"p{b0}", bufs=4)
                    for m in range(BLOCK):
                        j = b0 * BLOCK + m
                        vs = vs0_t[:, (BLOCK * j - m) * 2:
                                   (BLOCK * (j + 1) - m) * 2]
                        nc.tensor.matmul(
                            p_t[pb:pb + 32, 0:BATCH],
                            vs.bitcast(f16),
                            g_ts[0][:, j - _GSTART[gi], :],
                            start=(m == 0),
                            stop=False,
                            tile_position=(0, pb),
                        )
                    for m in range(BLOCK):
                        j = b0 * BLOCK + m
                        vs = vs1_t[:, :, BLOCK * j - m:BLOCK * (j + 1) - m]
                        nc.tensor.matmul(
                            p_t[pb:pb + 32, 0:BATCH],
                            vs.bitcast(f8),
                            g_ts[1][:, :, j - _GSTART[gi], :],
                            start=False,
                            stop=(m == BLOCK - 1),
                            perf_mode=mybir.MatmulPerfMode.DoubleRow,
                            tile_position=(0, pb),
                        )
                    # Lane-parallel fused tanh(psum / VSCALE) from PSUM.
                    nc.scalar.activation(
                        fin_t[:, b0], p_t[0:32, 0:BATCH],
                        mybir.ActivationFunctionType.Tanh,
                        scale=1.0 / VSCALE,
                    )
            # One output DMA: it waits only on the final block's ACT.
            nc.scalar.dma_start(out_d[:], fin_t[:])
    nc.compile()
    return nc


def _prepare(x, kernel_vector, bias, nonzero_ind):
    """Host-side shard prep. Returns (ks, per-core input dicts)."""
    x = np.asarray(x, dtype=np.float32)
    v = np.asarray(kernel_vector, dtype=np.float32).ravel()
    bias = np.asarray(bias, dtype=np.float32).ravel()
    ind = np.asarray(nonzero_ind)
    r = ind[:, 0].astype(np.int64)
    c = ind[:, 1].astype(np.int64)

    # COO .set semantics: de-duplicate (row, col), keeping the last occurrence.
    flat = r * UNITS + c
    if len(np.unique(flat)) != len(flat):
        _, last_rev = np.unique(flat[::-1], return_index=True)
        keep = np.sort(len(flat) - 1 - last_rev)
        r, c, v = r[keep], c[keep], v[keep]

    xt16 = np.ascontiguousarray(x.T).astype(np.float16)  # [INPUT_DIM, BATCH]

    # Sort by (column, |v| desc); slot k within column = |v| rank.
    order = np.lexsort((-np.abs(v), c))
    r_s, c_s, v_s = r[order], c[order], v[order]
    counts = np.bincount(c_s, minlength=UNITS)
    ks = _chunk_sizes(int(counts.max()))
    kp = 1 + C0_ENTRIES + sum(ks[1:])  # dense slot space incl bias at 127
    starts = np.zeros(UNITS + 1, dtype=np.int64)
    np.cumsum(counts, out=starts[1:])
    k_s = np.arange(len(c_s), dtype=np.int64) - starts[c_s]
    # entry slot: rank<127 -> slot=rank (chunk 0); else slot=rank+1
    slot = np.where(k_s < C0_ENTRIES, k_s, k_s + 1)

    vs_scaled = (v_s * VSCALE).astype(np.float32)
    val_all = np.zeros((UNITS, kp), dtype=np.float32)
    val_all[c_s, slot] = vs_scaled
    val_all[:, C0_ENTRIES] = bias * VSCALE
    g_all = np.zeros((UNITS, kp, BATCH), dtype=np.float16)
    g_all[c_s, slot] = xt16[r_s]
    g_all[:, C0_ENTRIES] = 1.0

    g_all = g_all.reshape(N_CORES, UNITS_PER_CORE, kp, BATCH)
    val_all = val_all.reshape(N_CORES, UNITS_PER_CORE, kp)

    f8 = ml_dtypes.float8_e4m3
    assert len(ks) == 2
    KD = (ks[1] + 1) // 2
    vbytes = UNITS_PER_CORE * 4
    in_maps = []
    for d in range(N_CORES):
        m = {}
        vpack = np.zeros((128, vbytes), dtype=np.uint8)
        # fp16 chunk
        gc0 = g_all[d, :, :ks[0]].astype(np.float16)  # [col, k, b]
        # fp8 tail, zero-padded to 2*KD slots, as [col, ktile, KD, b]
        gc1 = np.zeros((UNITS_PER_CORE, 2 * KD, BATCH), np.float32)
        gc1[:, :ks[1]] = g_all[d, :, ks[0]:].astype(np.float32)
        gc1 = gc1.reshape(UNITS_PER_CORE, 2, KD, BATCH).astype(f8)
        for gi, ncols in enumerate(COL_GROUPS):
            cs = slice(_GSTART[gi], _GSTART[gi] + ncols)
            m[f"g0_{gi}"] = np.ascontiguousarray(gc0[cs].transpose(1, 0, 2))
            m[f"g1_{gi}"] = np.ascontiguousarray(
                gc1[cs].transpose(2, 1, 0, 3))
        v0 = np.ascontiguousarray(
            val_all[d, :, :ks[0]].T).astype(np.float16)
        vpack[:ks[0], :2 * UNITS_PER_CORE] = v0.view(np.uint8)
        v1 = np.zeros((UNITS_PER_CORE, 2 * KD), np.float32)
        v1[:, :ks[1]] = val_all[d, :, ks[0]:]
        v1 = v1.reshape(UNITS_PER_CORE, 2, KD).astype(f8)
        # [col, kt, pos] -> [pos, kt, col]
        vpack[:KD, 2 * UNITS_PER_CORE:] = np.ascontiguousarray(
            v1.transpose(2, 1, 0)).reshape(KD, 2 * UNITS_PER_CORE).view(
                np.uint8)
        m["vpack"] = vpack
        in_maps.append(m)
    return ks, in_maps


def _unscramble(res):
    """[core][part, block, b] -> [32, 2048] f32. Column j at [j%32, j//32]."""
    nblk = UNITS_PER_CORE // 32
    out = np.empty((UNITS, BATCH), dtype=np.float32)
    jmap = (np.arange(32)[:, None] + 32 * np.arange(nblk)[None, :])
    for d in range(N_CORES):
        o = res.results[d]["out"].reshape(32, nblk, BATCH)
        out[d * UNITS_PER_CORE + jmap.ravel()] = o.reshape(-1, BATCH)
    return np.ascontiguousarray(out.T)


def _run(inputs, trace=False):
    from concourse.bass_utils import run_bass_kernel_spmd

    ks, in_maps = _prepare(**inputs)
    if ks not in _PROGRAM_CACHE:
        _PROGRAM_CACHE[ks] = _build_program(ks)
    nc = _PROGRAM_CACHE[ks]
    res = None
    for attempt in range(3):
        try:
            res = run_bass_kernel_spmd(
                nc, in_maps, list(range(N_CORES)), trace=trace,
            )
            break
        except Exception:
            # Transient device faults (e.g. NRT_EXEC_UNIT_UNRECOVERABLE)
            # clear on re-execution; re-raise only if persistent.
            if attempt == 2:
                raise
    assert res is not None
    return _unscramble(res), res


def kernel(**inputs):
    out, _ = _run(inputs, trace=False)
    return out
